# revision 13
# baseline (speedup 1.0000x reference)
"""TRN2 Bass kernel for nn_ConceptEmbeddingConceptPred.

Computes y = concat([einsum('bjd,ijd->bi', x, W_emb) + b_loo,
                     einsum('bjd,hjd->bh', x, W_full) + b_full], axis=1)
where W_emb is the leave-one-out scatter-embedding of W_loo (zero diagonal).

Flattened, this is a (4096 x 16384) @ (16384 x 136) GEMM.

Distribution: contraction(k)-parallel over the 8 cores — core c owns
concepts j in [16c, 16c+16) (k-slice of 2048). Each core computes a full
(136, 4096) partial product; partials are summed on the host (cheap),
bias added, transposed, concatenated.

v5 dataflow (fp16 operands, fp32 PSUM accumulate, fp16 outputs):
  - x is transposed and cast to fp16 on the host, so k sits on SBUF
    partitions straight off the DMA — no on-device transposes. x tiles
    are [128, 2048] (4 KB per-partition segments, full DMA rate).
  - kt-outer matmul order: the 4 batch chunks of a round share one
    stationary load, so the ~96 ns LDWEIGHTS penalty amortizes 4x.
    fp16 matmuls sustain 216 ns per 512-col stream (PE input-bus floor).
  - the 4 full-probe accumulators of a round are packed into ONE PSUM
    bank at col-group partition offsets {0,32,64,96}, freeing banks for
    loo double-buffering and collapsing the epilogue to one copy.
  - ~40 dummy 8-col matmuls at t=0 ramp the PE p-state while the first
    x tiles stream in; round r+1's x DMAs are enqueued before round r's
    output DMAs to avoid head-of-line blocking on the queues.
"""

import sys

for _p in ("/opt/trn_rl_repo",):
    if _p not in sys.path:
        sys.path.append(_p)

import numpy as np
import concourse.bacc as bacc
import concourse.mybir as mybir
import concourse.tile as tile
from concourse.bass_utils import run_bass_kernel_spmd

dt = mybir.dt

B, C, D, H = 4096, 128, 128, 8
NCORES = 8
JPC = C // NCORES  # 16 concept (= k) tiles per core
KPC = JPC * D  # 2048 contraction elements per core
BCHUNK = 512  # batch per PSUM accumulation chunk (fp32 bank limit)
NCH = 4  # chunks per round
RCHUNK = BCHUNK * NCH  # 2048 batch cols per round
NR = B // RCHUNK  # 2 rounds
NWARM = 40  # dummy 128-col matmuls to ramp the PE p-state during DMA fill

_nc_cache = None


def _build():
    global _nc_cache
    if _nc_cache is not None:
        return _nc_cache

    nc = bacc.Bacc(
        "TRN2", target_bir_lowering=False, debug=False, num_devices=NCORES
    )
    xt_d = nc.dram_tensor("x_t", (KPC, B), dt.float16, kind="ExternalInput").ap()
    wl_d = nc.dram_tensor(
        "w_loo_t", (D, JPC, C), dt.float16, kind="ExternalInput"
    ).ap()
    wf_d = nc.dram_tensor(
        "w_full_t", (D, JPC, H), dt.float16, kind="ExternalInput"
    ).ap()
    yl_d = nc.dram_tensor("y_loo_t", (C, B), dt.float16, kind="ExternalOutput").ap()
    # full-probe outputs stay in packed col-group layout: rows {32c..32c+7}
    # of A hold chunk c in {0,1,2}; B holds chunk 3 (PE quadrant 3 is
    # unusable, so only col-group bases 0/32/64 exist)
    yfa_d = nc.dram_tensor(
        "y_full_a", (128, NR, BCHUNK), dt.float16, kind="ExternalOutput"
    ).ap()
    yfb_d = nc.dram_tensor(
        "y_full_b", (H, NR, BCHUNK), dt.float16, kind="ExternalOutput"
    ).ap()

    with tile.TileContext(nc) as tc:
        with (
            tc.tile_pool(name="wpool", bufs=1) as wpool,
            tc.tile_pool(name="xpool", bufs=32) as xpool,
            tc.tile_pool(name="ypool", bufs=8) as ypool,
            tc.tile_pool(name="psl", bufs=5, space="PSUM") as psl,
            tc.tile_pool(name="psf", bufs=1, space="PSUM") as psf,
            tc.tile_pool(name="psw", bufs=1, space="PSUM") as psw,
        ):
            wl = wpool.tile([D, JPC, C], dt.float16)
            wf = wpool.tile([D, JPC, H], dt.float16)
            # wf (64 KB) leads the sync queue; wl (512 KB) is issued after
            # kt0's x slices so it doesn't delay the first matmul group
            nc.sync.dma_start(wf[:], wf_d[:])

            # PE p-state warmup: back-to-back 512-col matmuls on memset
            # tiles — no DMA dependency, so they start within ~1 us and
            # keep the array continuously busy until the first x tile lands
            warm_w = wpool.tile([128, H], dt.float16)
            warm_x = wpool.tile([128, 128], dt.float16)
            nc.vector.memset(warm_w[:], 0.25)
            nc.vector.memset(warm_x[:], 0.25)
            warm_ps = psw.tile([H, BCHUNK], dt.float32)
            for _ in range(NWARM):
                nc.tensor.matmul(
                    warm_ps[:, :128], warm_w[:], warm_x[:], start=True, stop=True
                )

            def issue_x(r, split_head=0, start_kt=0):
                # half-tile (and for the head, quarter-tile) DMA grain: the
                # 16 DMA engines fair-share everything outstanding, so with
                # whole-tile DMAs completion order skews to the back of the
                # ring; finer grain keeps completions tracking issue order
                c0 = r * RCHUNK
                xts = [None] * start_kt
                for kt in range(start_kt, JPC):
                    xn = xpool.tile(
                        [128, RCHUNK], dt.float16, tag="xn", name=f"xn_{r}_{kt}"
                    )
                    eng = nc.sync if kt % 2 == 0 else nc.scalar
                    npc = NCH if kt < split_head else 2
                    w = RCHUNK // npc
                    for q in range(npc):
                        eng.dma_start(
                            xn[:, q * w : (q + 1) * w],
                            xt_d[
                                kt * 128 : (kt + 1) * 128,
                                c0 + q * w : c0 + (q + 1) * w,
                            ],
                        )
                    xts.append(xn)
                return xts

            # round 0, kt0/kt1: fine-grained slices so the first chunks land
            # fast; wl rides sync between kt0 and the remaining tiles
            xts0 = []
            for kt in range(2):
                xn = xpool.tile(
                    [128, RCHUNK], dt.float16, tag="xn", name=f"xn_0_{kt}"
                )
                eng = nc.sync if kt % 2 == 0 else nc.scalar
                edges = [0, 256, 512, 1024, 1536, 2048]
                for a, b in zip(edges, edges[1:]):
                    eng.dma_start(xn[:, a:b], xt_d[kt * 128 : (kt + 1) * 128, a:b])
                xts0.append(xn)
                if kt == 0:
                    nc.sync.dma_start(wl[:], wl_d[:])
            xts = xts0 + issue_x(0, start_kt=2)[2:]
            for r in range(NR):
                accs_l = [
                    psl.tile([C, BCHUNK], dt.float32, tag="accl", name=f"accl{c}")
                    for c in range(NCH)
                ]
                acc_fa = psf.tile([128, BCHUNK], dt.float32, tag="accfa")
                acc_fb = psf.tile([H, BCHUNK], dt.float32, tag="accfb")
                # kt-outer: the 4 chunk matmuls share one stationary load
                for kt in range(JPC):
                    for c in range(NCH):
                        dst = (
                            acc_fa[32 * c : 32 * c + H, :]
                            if c < 3
                            else acc_fb[:]
                        )
                        nc.tensor.matmul(
                            dst,
                            wf[:, kt, :],
                            xts[kt][:, c * BCHUNK : (c + 1) * BCHUNK],
                            start=(kt == 0),
                            stop=(kt == JPC - 1),
                        )
                    for c in range(NCH):
                        nc.tensor.matmul(
                            accs_l[c][:],
                            wl[:, kt, :],
                            xts[kt][:, c * BCHUNK : (c + 1) * BCHUNK],
                            start=(kt == 0),
                            stop=(kt == JPC - 1),
                        )

                # next round's x DMAs enqueue BEFORE this round's output DMAs
                if r + 1 < NR:
                    xts_next = issue_x(r + 1)

                last = r == NR - 1
                for c in range(NCH):
                    bc = r * NCH + c
                    yl_sb = ypool.tile([C, BCHUNK], dt.float16, tag="yl")
                    if last and c % 2 == 1:
                        nc.scalar.copy(yl_sb[:], accs_l[c][:])
                    else:
                        nc.vector.tensor_copy(yl_sb[:], accs_l[c][:])
                    oeng = nc.sync if c < 2 else nc.scalar
                    oeng.dma_start(
                        yl_d[:, bc * BCHUNK : (bc + 1) * BCHUNK], yl_sb[:]
                    )
                yfa_sb = ypool.tile([128, BCHUNK], dt.float16, tag="yfa")
                nc.vector.tensor_copy(yfa_sb[:], acc_fa[:])
                nc.sync.dma_start(yfa_d[:, r, :], yfa_sb[:])
                yfb_sb = ypool.tile([H, BCHUNK], dt.float16, tag="yfb")
                if last:
                    nc.scalar.copy(yfb_sb[:], acc_fb[:])
                else:
                    nc.vector.tensor_copy(yfb_sb[:], acc_fb[:])
                nc.scalar.dma_start(yfb_d[:, r, :], yfb_sb[:])

                if r + 1 < NR:
                    xts = xts_next

    nc.compile()
    _nc_cache = nc
    return nc


def _embed_loo_weights(W_loo):
    # probe i sees concepts j != i; scatter into (C, C, D) with zero row at j=i
    I = np.arange(C)[:, None]
    J = np.arange(C)[None, :]
    src = np.clip(J - (J > I).astype(np.int64), 0, C - 2)  # (C, C)
    W_emb = np.take_along_axis(W_loo, src[:, :, None], axis=1)  # (C, C, D)
    return W_emb * (J != I)[:, :, None].astype(W_loo.dtype)


def _prep_in_maps(x, W_loo, W_full):
    x16 = np.asarray(x, dtype=np.float32).astype(np.float16)
    # (C, D, B) so each core's (JPC, D, B) k-slice is a contiguous view
    xt_all = np.ascontiguousarray(x16.transpose(1, 2, 0))
    W_emb = _embed_loo_weights(np.asarray(W_loo, dtype=np.float32))
    W_full = np.asarray(W_full, dtype=np.float32)
    in_maps = []
    for c in range(NCORES):
        jsl = slice(c * JPC, (c + 1) * JPC)
        xt_c = xt_all[jsl].reshape(KPC, B)
        # stationary layouts: [d, kt, out] so K (=d) is the partition dim
        wl_c = np.ascontiguousarray(
            W_emb[:, jsl, :].transpose(2, 1, 0).astype(np.float16)
        )
        wf_c = np.ascontiguousarray(
            W_full[:, jsl, :].transpose(2, 1, 0).astype(np.float16)
        )
        in_maps.append({"x_t": xt_c, "w_loo_t": wl_c, "w_full_t": wf_c})
    return in_maps


def _assemble(results, b_loo, b_full):
    y_loo_t = np.zeros((C, B), np.float64)
    y_full_t = np.zeros((H, B), np.float64)
    for res in results:
        y_loo_t += res["y_loo_t"]
        yf_a = res["y_full_a"]  # (128, NR, BCHUNK) packed col-groups 0..2
        yf_b = res["y_full_b"]  # (H, NR, BCHUNK) chunk 3
        for r in range(NR):
            for c in range(NCH):
                bc = r * NCH + c
                src = yf_a[32 * c : 32 * c + H, r, :] if c < 3 else yf_b[:, r, :]
                y_full_t[:, bc * BCHUNK : (bc + 1) * BCHUNK] += src
    y_loo = (y_loo_t.T + np.asarray(b_loo, np.float64)[None, :]).astype(np.float32)
    y_full = (y_full_t.T + np.asarray(b_full, np.float64)[None, :]).astype(np.float32)
    return np.concatenate([y_loo, y_full], axis=1)


def run_spmd(x, W_loo, b_loo, W_full, b_full, trace=False):
    nc = _build()
    in_maps = _prep_in_maps(x, W_loo, W_full)
    res = run_bass_kernel_spmd(
        nc, in_maps, core_ids=list(range(NCORES)), trace=trace
    )
    return _assemble(res.results, b_loo, b_full), res


def kernel(x, W_loo, b_loo, W_full, b_full):
    out, _ = run_spmd(x, W_loo, b_loo, W_full, b_full)
    return out
